# revision 30
# baseline (speedup 1.0000x reference)
"""Causal multi-head self-attention (B=2, S=2048, D=768, H=12) on 8 TRN2 NeuronCores.

Sharding: core c = (batch b=c//4, head-group hg=c%4 of 3 heads).
Each core computes Q/K/V for its 3 heads, causal attention, and the partial
output projection sum_h out_h @ Wo[:, h]^T -> (S, D) in fp16. Host sums the
4 head-group partials per batch in f32 (the unshard step).

v3 design:
  - all-fp16 datapath: X/W quantized to fp16 on host, output partials fp16
    (halves DMA in both directions; fp16 matmuls run 1 cycle/row, FWL
    halves LDWEIGHTS).
  - phase A (QKV^T) runs first with triple-buffered PSUM chains and
    coarse-grained DMAs; evictions split between DVE and ACT so neither
    engine binds the phase.
  - phase C processes k-tile PAIRS (t even on PE row-strip 0-63, t odd on
    strip 64-127) as row-tiled concurrent matmuls (contract dim is DK=64);
    Q/K live in both partition halves via SBUF-SBUF dup DMAs. Score tiles
    are double-buffered ([128,1024] x 2 = 4 banks) so ACT exp never waits
    on the next score matmul.
  - causal mask applied multiplicatively on the fp16 expt tile (DVE 4x).
  - denominator reciprocal via reciprocal_approx_fast (~5x faster).
  - head order h2, h0, h1: h1's per-q-chunk divide triggers the output
    projection immediately, so proj overlaps h1's attention (PSUM: scores
    4 banks + pouts 2 + proj 2 = 8).

On-core dataflow (transposed (feature, seq) layout):
  A) QKV^T: psum[m, s] += WcatT[i, m].T @ XT[i, s]; chunk packing
     m0=[q2;k2] m1=[v2;v0] m2=[q0;k0] m3=[q1;k1] m4=[v1;pad]
  B) V natural: PE-transpose V^T tiles -> vp = [V | ones], batched 4/PSUM tile
  C) per head, per k-tile pair: scoresT[k, q] = KT[:,t].T @ QT (causally
     valid windows only), exp on ACT -> fp16, triangular mask mult on the
     diagonal window, PV: pout[qc] += vp[t].T @ expT (65 rows: 64 data +
     denominator); per qc: recip(den) -> partition_broadcast -> mul
  D) projection per qc: psum[q, j] += octT[h, q].T @ WoT[h, j]; ACT copy
     fp16 (split 512+256 for pipelining); DMA out.
"""

import numpy as np
from contextlib import ExitStack

import concourse.bass as bass
import concourse.tile as tile
from concourse import bacc, mybir
from concourse import bass_utils

F32 = mybir.dt.float32
FP16 = mybir.dt.float16
AF = mybir.ActivationFunctionType

B, S, D, H = 2, 2048, 768, 12
DK = 64
HPC = 3            # heads per core
NCORES = 8
NI = D // 128      # 6 input-feature chunks
NM = 5             # output m-chunks of 128 (640 rows incl. 64 pad)
NT = S // 128      # 16 k-tiles
NQC = S // 512     # 4 q-chunks

# local head -> qk16 chunk of Q / of K; data lives in BOTH partition halves
QC = {0: 2, 1: 4, 2: 0}
KC = {0: 3, 1: 5, 2: 1}
# local head -> (base_partition, chunk) in the vt (V^T staging) buffer
VTPOS = {2: (0, 0), 0: (64, 0), 1: (0, 1)}

_NC_CACHE = {}


def build_nc(dbg=False):
    key = ("nc", dbg)
    if key in _NC_CACHE:
        return _NC_CACHE[key]
    nc = bacc.Bacc("TRN2", target_bir_lowering=False, debug=False,
                   num_devices=NCORES)

    xt_d = nc.dram_tensor("xt", [NI, 128, S], FP16, kind="ExternalInput").ap()
    wcat_d = nc.dram_tensor("wcat", [NI, 128, NM * 128], FP16, kind="ExternalInput").ap()
    wot_d = nc.dram_tensor("wot", [2, 128, D], FP16, kind="ExternalInput").ap()
    tri_d = nc.dram_tensor("trimask", [128, 128], FP16, kind="ExternalInput").ap()
    id_d = nc.dram_tensor("ident", [128, 128], FP16, kind="ExternalInput").ap()
    ones_d = nc.dram_tensor("vones", [128, HPC * NT], FP16, kind="ExternalInput").ap()
    out_d = nc.dram_tensor("out", [S, D], FP16, kind="ExternalOutput").ap()
    if dbg:
        qk_dbg = nc.dram_tensor("qk_dbg", [128, 6, S], FP16, kind="ExternalOutput").ap()
        vp_dbg = nc.dram_tensor("vp_dbg", [128, HPC, NT, DK + 1], FP16, kind="ExternalOutput").ap()
        oct_dbg = nc.dram_tensor("oct_dbg", [128, 2, S], FP16, kind="ExternalOutput").ap()
        ex_dbg = nc.dram_tensor("ex_dbg", [128, 4, 1024], FP16, kind="ExternalOutput").ap()

    with tile.TileContext(nc) as tc, ExitStack() as ctx:
        const = ctx.enter_context(tc.tile_pool(name="const", bufs=1))

        # persistent SBUF buffers (all fp16)
        xt = const.tile([128, NI, S], FP16)             # X^T
        wcat = const.tile([128, NI, NM * 128], FP16)    # W^T (QKV packed)
        wot = const.tile([128, 2, D], FP16)             # Wo^T [h0;h1],[h2;h2]
        trim = const.tile([128, 128], FP16)             # triangular 0/1 mask
        ident = const.tile([128, 128], FP16)
        qk16 = const.tile([128, 6, S], FP16)            # Q/K, both halves
        vt = const.tile([128, 2, S], FP16)              # V^T staging
        vp = const.tile([128, HPC, NT, DK + 1], FP16)   # V' = [V | ones]
        oct_ = const.tile([128, 2, S], FP16)            # packed out^T

        # coarse DMAs: one transfer for the first quarter of X columns plus
        # one for all weights, so the first phase-A chain starts ~10us in
        xtr = xt_d.rearrange("i p s -> p i s")
        nc.sync.dma_start(xt[:, :, 0:512], xtr[:, :, 0:512])
        nc.sync.dma_start(wcat[:], wcat_d.rearrange("i p f -> p i f"))
        nc.sync.dma_start(ident[:], id_d)
        nc.sync.dma_start(trim[:], tri_d)
        nc.sync.dma_start(vp[:, :, :, DK:DK + 1],
                          ones_d.rearrange("p (h t) -> p h t", h=HPC))
        nc.sync.dma_start(xt[:, :, 512:S], xtr[:, :, 512:S])
        nc.sync.dma_start(wot[:], wot_d.rearrange("c p f -> p c f"))

        sb_exp = ctx.enter_context(tc.tile_pool(name="sb_exp", bufs=1))
        sb_div = ctx.enter_context(tc.tile_pool(name="sb_div", bufs=1))

        # ---- phase A ----------------------------------------------------
        def evict(m, pq, sc):
            """PSUM -> fp16 SBUF eviction, lo half on DVE, hi half on ACT"""
            s0, s1 = sc * 512, (sc + 1) * 512
            lo, hi = None, None
            if m == 0:    # [q2; k2]
                lo, hi = qk16[0:64, 0, s0:s1], qk16[64:128, 1, s0:s1]
            elif m == 1:  # [v2; v0]
                lo, hi = vt[0:64, 0, s0:s1], vt[64:128, 0, s0:s1]
            elif m == 2:  # [q0; k0]
                lo, hi = qk16[0:64, 2, s0:s1], qk16[64:128, 3, s0:s1]
            elif m == 3:  # [q1; k1]
                lo, hi = qk16[0:64, 4, s0:s1], qk16[64:128, 5, s0:s1]
            else:         # [v1; pad]
                lo = vt[0:64, 1, s0:s1]
            nc.vector.tensor_copy(lo, pq[0:64, :])
            if hi is not None:
                nc.scalar.copy(hi, pq[64:128, :])

        def dup(chunk, src_lo):
            """copy one 64-partition half of a qk16 chunk to the other half"""
            if src_lo:
                nc.sync.dma_start(qk16[64:128, chunk, :], qk16[0:64, chunk, :])
            else:
                nc.sync.dma_start(qk16[0:64, chunk, :], qk16[64:128, chunk, :])

        with tc.tile_pool(name="ps_a", bufs=1, space="PSUM") as ps_a:
            def chainA(m, sc):
                pq = ps_a.tile([128, 512], F32, tag="pa", bufs=3,
                               name=f"pa{m}_{sc}")
                for i in range(NI):
                    nc.tensor.matmul(
                        pq[:], wcat[:, i, m * 128:(m + 1) * 128],
                        xt[:, i, sc * 512:(sc + 1) * 512],
                        start=(i == 0), stop=(i == NI - 1))
                evict(m, pq, sc)

            def trbatch(h, t0):
                """4 PE transposes into one PSUM tile, one batched vp copy"""
                vb, vc = VTPOS[h]
                ptr = ps_a.tile([128, 4 * DK], FP16, tag="tr", bufs=2,
                                name=f"tr{h}_{t0}")
                for k in range(4):
                    t = t0 + k
                    nc.tensor.transpose(
                        ptr[:, k * DK:(k + 1) * DK],
                        vt[vb:vb + DK, vc, t * 128:(t + 1) * 128],
                        ident[vb:vb + DK, vb:vb + DK])
                nc.vector.tensor_copy(vp[:, h, t0:t0 + 4, 0:DK], ptr[:])

            for sc in range(NQC):
                for m in range(NM):
                    chainA(m, sc)
                trbatch(2, 4 * sc)   # v2 tiles for this column range
            for c, src_lo in ((0, True), (1, False), (2, True), (3, False),
                              (4, True), (5, False)):
                dup(c, src_lo)
            for h in (0, 1):
                for t0 in range(0, NT, 4):
                    trbatch(h, t0)


        # ---- phase C ----------------------------------------------------
        def attn(h, ps_sc, scr_bufs, ps_o, po_bufs, on_divide=None, split=False):
            qc_ = QC[h]
            kc_ = KC[h]
            pouts = {}

            def scores(qp, t):
                strip = 0 if t % 2 == 0 else 64
                qcs = (2 * qp, 2 * qp + 1)
                qc_lo = t // 4
                off = 128 * (t % 4)
                pscr = ps_sc.tile([128, 1024], F32, tag="scr", bufs=scr_bufs,
                                 name=f"sc{h}_{qp}_{t}")
                for half, qc in enumerate(qcs):
                    if qc < qc_lo:
                        continue
                    cs = off if qc == qc_lo else 0
                    nc.tensor.matmul(
                        pscr[:, half * 512 + cs:(half + 1) * 512],
                        qk16[strip:strip + DK, kc_, t * 128:(t + 1) * 128],
                        qk16[strip:strip + DK, qc_,
                             qc * 512 + cs:(qc + 1) * 512],
                        start=True, stop=True)
                lo = (512 if qc_lo == qcs[1] else 0) + \
                     (off if qc_lo in qcs else 0)
                expt = sb_exp.tile([128, 1024], FP16, tag=f"ex{t % 2}",
                                   bufs=2, name=f"ex{h}_{qp}_{t}")
                nc.scalar.activation(expt[:, lo:1024], pscr[:, lo:1024], AF.Exp)
                if qc_lo in qcs:
                    w0 = (qc_lo - 2 * qp) * 512 + off
                    nc.vector.tensor_mul(expt[:, w0:w0 + 128],
                                         expt[:, w0:w0 + 128], trim[:, 0:128])
                if dbg and h == 2 and qp == 0 and t < 4:
                    nc.sync.dma_start(ex_dbg[:, t, :], expt[:])
                return expt

            def pv(qp, t, expt):
                qcs = (2 * qp, 2 * qp + 1)
                qc_lo = t // 4
                off = 128 * (t % 4)
                for half, qc in enumerate(qcs):
                    if qc < qc_lo:
                        continue
                    cs = off if qc == qc_lo else 0
                    if split:
                        # k-split: both 64-row halves run concurrently on the
                        # two PE row strips, accumulating into separate banks
                        for s_, po in ((0, pouts[qc][0]), (64, pouts[qc][1])):
                            nc.tensor.matmul(
                                po[:, cs:512],
                                vp[s_:s_ + DK, h, t, :],
                                expt[s_:s_ + DK,
                                     half * 512 + cs:(half + 1) * 512],
                                start=(t == 0), stop=(t == 4 * qc + 3))
                    else:
                        nc.tensor.matmul(
                            pouts[qc][:, cs:512],
                            vp[:, h, t, :],
                            expt[:, half * 512 + cs:(half + 1) * 512],
                            start=(t == 0), stop=(t == 4 * qc + 3))

            def divide(qc):
                nout = sb_div.tile([DK + 1, 512], F32, tag="nout", bufs=2,
                                   name=f"no{h}_{qc}")
                if split:
                    nc.vector.tensor_copy(nout[:], pouts[qc][0][:])
                    nc.vector.tensor_add(nout[:], nout[:], pouts[qc][1][:])
                else:
                    nc.vector.tensor_copy(nout[:], pouts[qc][:])
                # the custom-DVE reciprocal misreads nonzero partition bases
                # on HW: shift the denominator row to partition 0 first
                drow = sb_div.tile([1, 512], F32, tag="drow", bufs=2,
                                   name=f"dr{h}_{qc}")
                nc.sync.dma_start(drow[:], nout[DK:DK + 1, :])
                rc = sb_div.tile([1, 512], F32, tag="rc", bufs=2,
                                 name=f"rc{h}_{qc}")
                nc.vector.reciprocal_approx_fast(rc[:], drow[:])
                rb = sb_div.tile([DK, 512], F32, tag="rb", bufs=2,
                                 name=f"rb{h}_{qc}")
                nc.gpsimd.partition_broadcast(rb[:], rc[:])
                qw = slice(qc * 512, (qc + 1) * 512)
                if h == 0:
                    nc.vector.tensor_mul(oct_[0:DK, 0, qw], nout[0:DK, :], rb[:])
                elif h == 1:   # lands at partitions 64-127: shift via DMA
                    tmp = sb_div.tile([DK, 512], FP16, tag="tmp", bufs=2,
                                      name=f"tmp{h}_{qc}")
                    nc.vector.tensor_mul(tmp[:], nout[0:DK, :], rb[:])
                    nc.sync.dma_start(oct_[DK:128, 0, qw], tmp[:])
                else:          # h2: chunk 1 lo
                    nc.vector.tensor_mul(oct_[0:DK, 1, qw], nout[0:DK, :], rb[:])
                if on_divide is not None:
                    on_divide(qc)

            prev = None
            for qp in range(2):
                for qc in (2 * qp, 2 * qp + 1):
                    if split:
                        pouts[qc] = (
                            ps_o.tile([DK + 1, 512], F32, tag="poutA",
                                      bufs=po_bufs, name=f"poA{h}_{qc}"),
                            ps_o.tile([DK + 1, 512], F32, tag="poutB",
                                      bufs=po_bufs, name=f"poB{h}_{qc}"),
                        )
                    else:
                        pouts[qc] = ps_o.tile([DK + 1, 512], F32, tag="pout",
                                              bufs=po_bufs,
                                              name=f"po{h}_{qc}")
                for t0 in range(0, 8 * qp + 8, 2):
                    e0 = scores(qp, t0)
                    e1 = scores(qp, t0 + 1)
                    if prev is not None:
                        pqp, pt0, pe0, pe1 = prev
                        pv(pqp, pt0, pe0)
                        pv(pqp, pt0 + 1, pe1)
                        if pt0 + 1 == 4 * (2 * pqp) + 3:
                            divide(2 * pqp)
                        elif pt0 + 1 == 4 * (2 * pqp + 1) + 3:
                            divide(2 * pqp + 1)
                    prev = (qp, t0, e0, e1)
                    yield
            pqp, pt0, pe0, pe1 = prev
            pv(pqp, pt0, pe0)
            pv(pqp, pt0 + 1, pe1)
            divide(2 * pqp + 1)

        # ---- output projection for one q-chunk; 512/256 split pipelines
        # the ACT eviction against the next tile's matmuls
        def proj(qc):
            for qt in range(4 * qc, 4 * qc + 4):
                octq = oct_[:, 0, qt * 128:(qt + 1) * 128]
                oct2 = oct_[0:DK, 1, qt * 128:(qt + 1) * 128]
                pp0 = ps_p.tile([128, 512], F32, tag="pp0", bufs=2,
                                name=f"pp0_{qt}")
                pp1 = ps_p.tile([128, 256], F32, tag="pp1", bufs=2,
                                name=f"pp1_{qt}")
                ot = sb_exp.tile([128, D], FP16, tag="ot", bufs=2,
                                 name=f"ot{qt}")
                nc.tensor.matmul(pp0[:], octq, wot[:, 0, 0:512],
                                 start=True, stop=False)
                nc.tensor.matmul(pp0[:], oct2, wot[0:DK, 1, 0:512],
                                 start=False, stop=True)
                nc.vector.tensor_copy(ot[:, 0:512], pp0[:])
                nc.tensor.matmul(pp1[:], octq, wot[:, 0, 512:D],
                                 start=True, stop=False)
                nc.tensor.matmul(pp1[:], oct2, wot[0:DK, 1, 512:D],
                                 start=False, stop=True)
                nc.scalar.copy(ot[:, 512:D], pp1[:])
                nc.sync.dma_start(out_d[qt * 128:(qt + 1) * 128, :], ot[:])

        # h2, h0 with k-split PV: even at HAM-cold PE clocks the matmuls fit
        # under the ACT exp cadence, so these segments run at ACT speed
        with tc.tile_pool(name="ps_s1", bufs=1, space="PSUM") as ps_s1, \
             tc.tile_pool(name="ps_o1", bufs=1, space="PSUM") as ps_o1:
            for _ in attn(2, ps_s1, 2, ps_o1, 2, split=True):
                pass
            for _ in attn(0, ps_s1, 2, ps_o1, 2, split=True):
                pass

        # h1 interleaved with the projection (launched per qc): the denser
        # PE stream keeps HAM warm through the tail
        # h1: single-buffered scores free 2 banks so proj double-buffers
        ps_s2 = ctx.enter_context(tc.tile_pool(name="ps_s2", bufs=1, space="PSUM"))
        ps_o2 = ctx.enter_context(tc.tile_pool(name="ps_o2", bufs=1, space="PSUM"))
        ps_p = ctx.enter_context(tc.tile_pool(name="ps_p", bufs=1, space="PSUM"))
        for _ in attn(1, ps_s2, 1, ps_o2, 2, on_divide=proj):
            pass

        if dbg:
            nc.sync.dma_start(qk_dbg, qk16[:])
            nc.sync.dma_start(vp_dbg, vp[:])
            nc.sync.dma_start(oct_dbg, oct_[:])

    nc.compile()
    _NC_CACHE[key] = nc
    return nc


def make_in_maps(X, Wq, Wk, Wv, Wo):
    X = np.ascontiguousarray(np.asarray(X, dtype=np.float32))
    Wq = np.asarray(Wq, dtype=np.float32)
    Wk = np.asarray(Wk, dtype=np.float32)
    Wv = np.asarray(Wv, dtype=np.float32)
    Wo = np.asarray(Wo, dtype=np.float32)

    # triangular keep-mask for the diagonal window: rows=k (p), cols=q;
    # keep iff q >= k
    p = np.arange(128)[:, None]
    c = np.arange(128)[None, :]
    trimask = (c >= p).astype(np.float16)
    ident = np.eye(128, dtype=np.float16)
    vones = np.ones((128, HPC * NT), dtype=np.float16)

    in_maps = []
    for core in range(NCORES):
        b, hg = core // 4, core % 4
        gh = [hg * HPC + l for l in range(HPC)]
        q = [Wq[g * DK:(g + 1) * DK, :] / 8.0 for g in gh]
        k = [Wk[g * DK:(g + 1) * DK, :] for g in gh]
        v = [Wv[g * DK:(g + 1) * DK, :] for g in gh]
        # m0=[q2;k2] m1=[v2;v0] m2=[q0;k0] m3=[q1;k1] m4=[v1;pad]
        wcat_rows = np.vstack([
            q[2], k[2], v[2], v[0], q[0], k[0], q[1], k[1], v[1],
            np.zeros((DK, D), dtype=np.float32),
        ])                                            # (640, 768)
        wcat = np.ascontiguousarray(
            wcat_rows.T.reshape(NI, 128, NM * 128)).astype(np.float16)
        w0, w1, w2 = (Wo[:, g * DK:(g + 1) * DK].T for g in gh)
        wot = np.ascontiguousarray(np.stack([
            np.vstack([w0, w1]), np.vstack([w2, w2]),
        ]).astype(np.float16))                        # (2, 128, 768)
        xt = np.ascontiguousarray(X[b].T.reshape(NI, 128, S)).astype(np.float16)
        in_maps.append({
            "xt": xt, "wcat": wcat, "wot": wot,
            "trimask": trimask, "ident": ident, "vones": vones,
        })
    return in_maps


def _run(in_maps, trace=False, trace_cores=None):
    nc = build_nc()
    return bass_utils.run_bass_kernel_spmd(
        nc, in_maps, core_ids=list(range(NCORES)),
        trace=trace, trace_cores=trace_cores,
    )


def kernel(X, Wq, Wk, Wv, Wo):
    in_maps = make_in_maps(X, Wq, Wk, Wv, Wo)
    res = _run(in_maps, trace=False)
    out = np.zeros((B, S, D), dtype=np.float32)
    for c in range(NCORES):
        out[c // 4] += res.results[c]["out"].astype(np.float32)
    return out


# revision 33
# speedup vs baseline: 1.1402x; 1.1402x over previous
"""Causal multi-head self-attention (B=2, S=2048, D=768, H=12) on 8 TRN2 NeuronCores.

Sharding: core c = (batch b=c//4, head-group hg=c%4 of 3 heads).
Each core computes Q/K/V for its 3 heads, causal attention, and the partial
output projection sum_h out_h @ Wo[:, h]^T -> (S, D) in fp16. Host sums the
4 head-group partials per batch in f32 (the unshard step).

v3 design:
  - all-fp16 datapath: X/W quantized to fp16 on host, output partials fp16
    (halves DMA in both directions; fp16 matmuls run 1 cycle/row, FWL
    halves LDWEIGHTS).
  - phase A (QKV^T) runs first with triple-buffered PSUM chains and
    coarse-grained DMAs; evictions split between DVE and ACT so neither
    engine binds the phase.
  - phase C processes k-tile PAIRS (t even on PE row-strip 0-63, t odd on
    strip 64-127) as row-tiled concurrent matmuls (contract dim is DK=64);
    Q/K live in both partition halves via SBUF-SBUF dup DMAs. Score tiles
    are double-buffered ([128,1024] x 2 = 4 banks) so ACT exp never waits
    on the next score matmul.
  - causal mask applied multiplicatively on the fp16 expt tile (DVE 4x).
  - denominator reciprocal via reciprocal_approx_fast (~5x faster).
  - head order h2, h0, h1: h1's per-q-chunk divide triggers the output
    projection immediately, so proj overlaps h1's attention (PSUM: scores
    4 banks + pouts 2 + proj 2 = 8).

On-core dataflow (transposed (feature, seq) layout):
  A) QKV^T: psum[m, s] += WcatT[i, m].T @ XT[i, s]; chunk packing
     m0=[q2;k2] m1=[v2;v0] m2=[q0;k0] m3=[q1;k1] m4=[v1;pad]
  B) V natural: PE-transpose V^T tiles -> vp = [V | ones], batched 4/PSUM tile
  C) per head, per k-tile pair: scoresT[k, q] = KT[:,t].T @ QT (causally
     valid windows only), exp on ACT -> fp16, triangular mask mult on the
     diagonal window, PV: pout[qc] += vp[t].T @ expT (65 rows: 64 data +
     denominator); per qc: recip(den) -> partition_broadcast -> mul
  D) projection per qc: psum[q, j] += octT[h, q].T @ WoT[h, j]; ACT copy
     fp16 (split 512+256 for pipelining); DMA out.
"""

import numpy as np
from contextlib import ExitStack

import concourse.bass as bass
import concourse.tile as tile
from concourse import bacc, mybir
from concourse import bass_utils

F32 = mybir.dt.float32
FP16 = mybir.dt.float16
AF = mybir.ActivationFunctionType

B, S, D, H = 2, 2048, 768, 12
DK = 64
HPC = 3            # heads per core
NCORES = 8
NI = D // 128      # 6 input-feature chunks
NM = 5             # output m-chunks of 128 (640 rows incl. 64 pad)
NT = S // 128      # 16 k-tiles
NQC = S // 512     # 4 q-chunks

# local head -> qk16 chunk of Q / of K; data lives in BOTH partition halves
QC = {0: 2, 1: 4, 2: 0}
KC = {0: 3, 1: 5, 2: 1}
# local head -> (base_partition, chunk) in the vt (V^T staging) buffer
VTPOS = {2: (0, 0), 0: (64, 0), 1: (0, 1)}

_NC_CACHE = {}


def build_nc(dbg=False):
    key = ("nc", dbg)
    if key in _NC_CACHE:
        return _NC_CACHE[key]
    nc = bacc.Bacc("TRN2", target_bir_lowering=False, debug=False,
                   num_devices=NCORES)

    xt_d = nc.dram_tensor("xt", [NI, 128, S], FP16, kind="ExternalInput").ap()
    wcat_d = nc.dram_tensor("wcat", [NI, 128, NM * 128], FP16, kind="ExternalInput").ap()
    wot_d = nc.dram_tensor("wot", [2, 128, D], FP16, kind="ExternalInput").ap()
    tri_d = nc.dram_tensor("trimask", [128, 128], FP16, kind="ExternalInput").ap()
    id_d = nc.dram_tensor("ident", [128, 128], FP16, kind="ExternalInput").ap()
    ones_d = nc.dram_tensor("vones", [128, HPC * NT], FP16, kind="ExternalInput").ap()
    out_d = nc.dram_tensor("out", [S, D], FP16, kind="ExternalOutput").ap()
    if dbg:
        qk_dbg = nc.dram_tensor("qk_dbg", [128, 6, S], FP16, kind="ExternalOutput").ap()
        vp_dbg = nc.dram_tensor("vp_dbg", [128, HPC, NT, DK + 1], FP16, kind="ExternalOutput").ap()
        oct_dbg = nc.dram_tensor("oct_dbg", [128, 2, S], FP16, kind="ExternalOutput").ap()
        ex_dbg = nc.dram_tensor("ex_dbg", [128, 4, 1024], FP16, kind="ExternalOutput").ap()

    with tile.TileContext(nc) as tc, ExitStack() as ctx:
        const = ctx.enter_context(tc.tile_pool(name="const", bufs=1))

        # persistent SBUF buffers (all fp16)
        xt = const.tile([128, NI, S], FP16)             # X^T
        wcat = const.tile([128, NI, NM * 128], FP16)    # W^T (QKV packed)
        wot = const.tile([128, 2, D], FP16)             # Wo^T [h0;h1],[h2;h2]
        trim = const.tile([128, 128], FP16)             # triangular 0/1 mask
        ident = const.tile([128, 128], FP16)
        qk16 = const.tile([128, 6, S], FP16)            # Q/K, both halves
        vt = const.tile([128, 2, S], FP16)              # V^T staging
        vp = const.tile([128, HPC, NT, 128], FP16)      # V' = [V | ones | 0pad]
        oct_ = const.tile([128, 2, S], FP16)            # packed out^T

        # zero vp's pad columns once so 128-col PV stationaries are FWL-
        # eligible; runs on DVE during the initial DMA wait
        nc.vector.memset(vp[:], 0.0)

        # coarse DMAs: one transfer for the first quarter of X columns plus
        # one for all weights, so the first phase-A chain starts ~10us in
        xtr = xt_d.rearrange("i p s -> p i s")
        nc.sync.dma_start(xt[:, :, 0:512], xtr[:, :, 0:512])
        nc.sync.dma_start(wcat[:], wcat_d.rearrange("i p f -> p i f"))
        nc.sync.dma_start(ident[:], id_d)
        nc.sync.dma_start(trim[:], tri_d)
        nc.sync.dma_start(vp[:, :, :, DK:DK + 1],
                          ones_d.rearrange("p (h t) -> p h t", h=HPC))
        nc.sync.dma_start(xt[:, :, 512:S], xtr[:, :, 512:S])
        nc.sync.dma_start(wot[:], wot_d.rearrange("c p f -> p c f"))

        sb_exp = ctx.enter_context(tc.tile_pool(name="sb_exp", bufs=1))
        sb_div = ctx.enter_context(tc.tile_pool(name="sb_div", bufs=1))

        # ---- phase A ----------------------------------------------------
        def evict(m, pq, sc):
            """PSUM -> fp16 SBUF eviction, lo half on DVE, hi half on ACT"""
            s0, s1 = sc * 512, (sc + 1) * 512
            lo, hi = None, None
            if m == 0:    # [q2; k2]
                lo, hi = qk16[0:64, 0, s0:s1], qk16[64:128, 1, s0:s1]
            elif m == 1:  # [v2; v0]
                lo, hi = vt[0:64, 0, s0:s1], vt[64:128, 0, s0:s1]
            elif m == 2:  # [q0; k0]
                lo, hi = qk16[0:64, 2, s0:s1], qk16[64:128, 3, s0:s1]
            elif m == 3:  # [q1; k1]
                lo, hi = qk16[0:64, 4, s0:s1], qk16[64:128, 5, s0:s1]
            else:         # [v1; pad]
                lo = vt[0:64, 1, s0:s1]
            nc.vector.tensor_copy(lo, pq[0:64, :])
            if hi is not None:
                nc.scalar.copy(hi, pq[64:128, :])

        def dup(chunk, src_lo):
            """copy one 64-partition half of a qk16 chunk to the other half"""
            if src_lo:
                nc.sync.dma_start(qk16[64:128, chunk, :], qk16[0:64, chunk, :])
            else:
                nc.sync.dma_start(qk16[0:64, chunk, :], qk16[64:128, chunk, :])

        with tc.tile_pool(name="ps_a", bufs=1, space="PSUM") as ps_a:
            def chainA(m, sc):
                pq = ps_a.tile([128, 512], F32, tag="pa", bufs=3,
                               name=f"pa{m}_{sc}")
                for i in range(NI):
                    nc.tensor.matmul(
                        pq[:], wcat[:, i, m * 128:(m + 1) * 128],
                        xt[:, i, sc * 512:(sc + 1) * 512],
                        start=(i == 0), stop=(i == NI - 1))
                evict(m, pq, sc)

            def trbatch(h, t0):
                """4 PE transposes into one PSUM tile, one batched vp copy"""
                vb, vc = VTPOS[h]
                ptr = ps_a.tile([128, 4 * DK], FP16, tag="tr", bufs=2,
                                name=f"tr{h}_{t0}")
                for k in range(4):
                    t = t0 + k
                    nc.tensor.transpose(
                        ptr[:, k * DK:(k + 1) * DK],
                        vt[vb:vb + DK, vc, t * 128:(t + 1) * 128],
                        ident[vb:vb + DK, vb:vb + DK])
                nc.vector.tensor_copy(vp[:, h, t0:t0 + 4, 0:DK], ptr[:])

            for sc in range(NQC):
                for m in range(NM):
                    chainA(m, sc)
                trbatch(2, 4 * sc)   # v2 tiles for this column range
            for c, src_lo in ((0, True), (1, False), (2, True), (3, False),
                              (4, True), (5, False)):
                dup(c, src_lo)
            for h in (0, 1):
                for t0 in range(0, NT, 4):
                    trbatch(h, t0)


        # ---- phase C ----------------------------------------------------
        def attn(h, ps_sc, scr_bufs, ps_o, po_bufs, on_divide=None, split=False):
            qc_ = QC[h]
            kc_ = KC[h]
            pouts = {}

            def scores(qp, t):
                strip = 0 if t % 2 == 0 else 64
                qcs = (2 * qp, 2 * qp + 1)
                qc_lo = t // 4
                off = 128 * (t % 4)
                pscr = ps_sc.tile([128, 1024], F32, tag="scr", bufs=scr_bufs,
                                 name=f"sc{h}_{qp}_{t}")
                for half, qc in enumerate(qcs):
                    if qc < qc_lo:
                        continue
                    cs = off if qc == qc_lo else 0
                    nc.tensor.matmul(
                        pscr[:, half * 512 + cs:(half + 1) * 512],
                        qk16[strip:strip + DK, kc_, t * 128:(t + 1) * 128],
                        qk16[strip:strip + DK, qc_,
                             qc * 512 + cs:(qc + 1) * 512],
                        start=True, stop=True)
                lo = (512 if qc_lo == qcs[1] else 0) + \
                     (off if qc_lo in qcs else 0)
                expt = sb_exp.tile([128, 1024], FP16, tag=f"ex{t % 2}",
                                   bufs=2, name=f"ex{h}_{qp}_{t}")
                nc.scalar.activation(expt[:, lo:1024], pscr[:, lo:1024], AF.Exp)
                if qc_lo in qcs:
                    w0 = (qc_lo - 2 * qp) * 512 + off
                    nc.vector.tensor_mul(expt[:, w0:w0 + 128],
                                         expt[:, w0:w0 + 128], trim[:, 0:128])
                if dbg and h == 2 and qp == 0 and t < 4:
                    nc.sync.dma_start(ex_dbg[:, t, :], expt[:])
                return expt

            def pv(qp, t, expt):
                qcs = (2 * qp, 2 * qp + 1)
                qc_lo = t // 4
                off = 128 * (t % 4)
                for half, qc in enumerate(qcs):
                    if qc < qc_lo:
                        continue
                    cs = off if qc == qc_lo else 0
                    if split:
                        # k-split: both 64-row halves run concurrently on the
                        # two PE row strips, accumulating into separate banks
                        for s_, po in ((0, pouts[qc][0]), (64, pouts[qc][1])):
                            nc.tensor.matmul(
                                po[:, cs:512],
                                vp[s_:s_ + DK, h, t, :],
                                expt[s_:s_ + DK,
                                     half * 512 + cs:(half + 1) * 512],
                                start=(t == 0), stop=(t == 4 * qc + 3))
                    else:
                        nc.tensor.matmul(
                            pouts[qc][:, cs:512],
                            vp[:, h, t, :],
                            expt[:, half * 512 + cs:(half + 1) * 512],
                            start=(t == 0), stop=(t == 4 * qc + 3))

            def divide(qc):
                nout = sb_div.tile([DK + 1, 512], F32, tag="nout", bufs=2,
                                   name=f"no{h}_{qc}")
                if split:
                    nc.vector.tensor_copy(nout[:], pouts[qc][0][0:DK + 1, :])
                    nc.vector.tensor_add(nout[:], nout[:], pouts[qc][1][0:DK + 1, :])
                else:
                    nc.vector.tensor_copy(nout[:], pouts[qc][0:DK + 1, :])
                # the custom-DVE reciprocal misreads nonzero partition bases
                # on HW: shift the denominator row to partition 0 first
                drow = sb_div.tile([1, 512], F32, tag="drow", bufs=2,
                                   name=f"dr{h}_{qc}")
                nc.sync.dma_start(drow[:], nout[DK:DK + 1, :])
                rc = sb_div.tile([1, 512], F32, tag="rc", bufs=2,
                                 name=f"rc{h}_{qc}")
                nc.vector.reciprocal_approx_fast(rc[:], drow[:])
                rb = sb_div.tile([DK, 512], F32, tag="rb", bufs=2,
                                 name=f"rb{h}_{qc}")
                nc.gpsimd.partition_broadcast(rb[:], rc[:])
                qw = slice(qc * 512, (qc + 1) * 512)
                if h == 0:
                    nc.vector.tensor_mul(oct_[0:DK, 0, qw], nout[0:DK, :], rb[:])
                elif h == 1:   # lands at partitions 64-127: shift via DMA
                    tmp = sb_div.tile([DK, 512], FP16, tag="tmp", bufs=2,
                                      name=f"tmp{h}_{qc}")
                    nc.vector.tensor_mul(tmp[:], nout[0:DK, :], rb[:])
                    nc.sync.dma_start(oct_[DK:128, 0, qw], tmp[:])
                else:          # h2: chunk 1 lo
                    nc.vector.tensor_mul(oct_[0:DK, 1, qw], nout[0:DK, :], rb[:])
                if on_divide is not None:
                    on_divide(qc)

            prev = None
            for qp in range(2):
                for qc in (2 * qp, 2 * qp + 1):
                    if split:
                        pouts[qc] = (
                            ps_o.tile([128, 512], F32, tag="poutA",
                                      bufs=po_bufs, name=f"poA{h}_{qc}"),
                            ps_o.tile([128, 512], F32, tag="poutB",
                                      bufs=po_bufs, name=f"poB{h}_{qc}"),
                        )
                    else:
                        pouts[qc] = ps_o.tile([128, 512], F32, tag="pout",
                                              bufs=po_bufs,
                                              name=f"po{h}_{qc}")
                for t0 in range(0, 8 * qp + 8, 2):
                    e0 = scores(qp, t0)
                    e1 = scores(qp, t0 + 1)
                    if prev is not None:
                        pqp, pt0, pe0, pe1 = prev
                        pv(pqp, pt0, pe0)
                        pv(pqp, pt0 + 1, pe1)
                        if pt0 + 1 == 4 * (2 * pqp) + 3:
                            divide(2 * pqp)
                        elif pt0 + 1 == 4 * (2 * pqp + 1) + 3:
                            divide(2 * pqp + 1)
                    prev = (qp, t0, e0, e1)
                    yield
            pqp, pt0, pe0, pe1 = prev
            pv(pqp, pt0, pe0)
            pv(pqp, pt0 + 1, pe1)
            divide(2 * pqp + 1)

        # ---- output projection for one q-chunk; 512/256 split pipelines
        # the ACT eviction against the next tile's matmuls
        def proj(qc):
            for qt in range(4 * qc, 4 * qc + 4):
                octq = oct_[:, 0, qt * 128:(qt + 1) * 128]
                oct2 = oct_[0:DK, 1, qt * 128:(qt + 1) * 128]
                pp0 = ps_p.tile([128, 512], F32, tag="pp0", bufs=1,
                                name=f"pp0_{qt}")
                pp1 = ps_p.tile([128, 256], F32, tag="pp1", bufs=1,
                                name=f"pp1_{qt}")
                ot = sb_exp.tile([128, D], FP16, tag="ot", bufs=2,
                                 name=f"ot{qt}")
                nc.tensor.matmul(pp0[:], octq, wot[:, 0, 0:512],
                                 start=True, stop=False)
                nc.tensor.matmul(pp0[:], oct2, wot[0:DK, 1, 0:512],
                                 start=False, stop=True)
                nc.vector.tensor_copy(ot[:, 0:512], pp0[:])
                nc.tensor.matmul(pp1[:], octq, wot[:, 0, 512:D],
                                 start=True, stop=False)
                nc.tensor.matmul(pp1[:], oct2, wot[0:DK, 1, 512:D],
                                 start=False, stop=True)
                nc.scalar.copy(ot[:, 512:D], pp1[:])
                nc.sync.dma_start(out_d[qt * 128:(qt + 1) * 128, :], ot[:])

        # h2, h0 with k-split PV: even at HAM-cold PE clocks the matmuls fit
        # under the ACT exp cadence, so these segments run at ACT speed
        with tc.tile_pool(name="ps_s1", bufs=1, space="PSUM") as ps_s1, \
             tc.tile_pool(name="ps_o1", bufs=1, space="PSUM") as ps_o1:
            for _ in attn(2, ps_s1, 2, ps_o1, 2, split=True):
                pass
            for _ in attn(0, ps_s1, 2, ps_o1, 2, split=True):
                pass

        # h1 interleaved with the projection (launched per qc): the denser
        # PE stream keeps HAM warm through the tail
        ps_s2 = ctx.enter_context(tc.tile_pool(name="ps_s2", bufs=1, space="PSUM"))
        ps_o2 = ctx.enter_context(tc.tile_pool(name="ps_o2", bufs=1, space="PSUM"))
        ps_p = ctx.enter_context(tc.tile_pool(name="ps_p", bufs=1, space="PSUM"))
        for _ in attn(1, ps_s2, 2, ps_o2, 2, on_divide=proj):
            pass

        if dbg:
            nc.sync.dma_start(qk_dbg, qk16[:])
            nc.sync.dma_start(vp_dbg, vp[:, :, :, 0:DK + 1])
            nc.sync.dma_start(oct_dbg, oct_[:])

    nc.compile()
    _NC_CACHE[key] = nc
    return nc


def make_in_maps(X, Wq, Wk, Wv, Wo):
    X = np.ascontiguousarray(np.asarray(X, dtype=np.float32))
    Wq = np.asarray(Wq, dtype=np.float32)
    Wk = np.asarray(Wk, dtype=np.float32)
    Wv = np.asarray(Wv, dtype=np.float32)
    Wo = np.asarray(Wo, dtype=np.float32)

    # triangular keep-mask for the diagonal window: rows=k (p), cols=q;
    # keep iff q >= k
    p = np.arange(128)[:, None]
    c = np.arange(128)[None, :]
    trimask = (c >= p).astype(np.float16)
    ident = np.eye(128, dtype=np.float16)
    vones = np.ones((128, HPC * NT), dtype=np.float16)

    in_maps = []
    for core in range(NCORES):
        b, hg = core // 4, core % 4
        gh = [hg * HPC + l for l in range(HPC)]
        q = [Wq[g * DK:(g + 1) * DK, :] / 8.0 for g in gh]
        k = [Wk[g * DK:(g + 1) * DK, :] for g in gh]
        v = [Wv[g * DK:(g + 1) * DK, :] for g in gh]
        # m0=[q2;k2] m1=[v2;v0] m2=[q0;k0] m3=[q1;k1] m4=[v1;pad]
        wcat_rows = np.vstack([
            q[2], k[2], v[2], v[0], q[0], k[0], q[1], k[1], v[1],
            np.zeros((DK, D), dtype=np.float32),
        ])                                            # (640, 768)
        wcat = np.ascontiguousarray(
            wcat_rows.T.reshape(NI, 128, NM * 128)).astype(np.float16)
        w0, w1, w2 = (Wo[:, g * DK:(g + 1) * DK].T for g in gh)
        wot = np.ascontiguousarray(np.stack([
            np.vstack([w0, w1]), np.vstack([w2, w2]),
        ]).astype(np.float16))                        # (2, 128, 768)
        xt = np.ascontiguousarray(X[b].T.reshape(NI, 128, S)).astype(np.float16)
        in_maps.append({
            "xt": xt, "wcat": wcat, "wot": wot,
            "trimask": trimask, "ident": ident, "vones": vones,
        })
    return in_maps


def _run(in_maps, trace=False, trace_cores=None):
    nc = build_nc()
    return bass_utils.run_bass_kernel_spmd(
        nc, in_maps, core_ids=list(range(NCORES)),
        trace=trace, trace_cores=trace_cores,
    )


def kernel(X, Wq, Wk, Wv, Wo):
    in_maps = make_in_maps(X, Wq, Wk, Wv, Wo)
    res = _run(in_maps, trace=False)
    out = np.zeros((B, S, D), dtype=np.float32)
    for c in range(NCORES):
        out[c // 4] += res.results[c]["out"].astype(np.float32)
    return out
